# revision 23
# baseline (speedup 1.0000x reference)
"""AR(16) Gaussian log-likelihood kernel for Trainium2, 8 NeuronCores.

Math: out[b, t] = C - ((s[b,t] - sum_{k=1..16} phi_k s[b,t-k]) * invsc)^2
  with C = -0.5*log(2*pi*sigma^2), invsc = 1/(sqrt(2)*sigma).

Strategy (pure data parallel, 32 rows per core):
  - View each core's [32, 65536] shard as 8-row tiles laid out on 128
    SBUF partitions with U = 4096 contiguous samples per partition,
    processed in half-tiles of H = 2048 samples (+64-sample halo).
  - Input DMA casts f32->bf16 (SWDGE). DVE stream-transposes the bf16
    data viewed as int32 pairs (halves transpose cost): partition a of a
    32-group then holds the sample pairs (64j + 2a, 64j + 2a + 1).
  - TensorE computes q = (pred - s)*invsc with 6 banded-Toeplitz matmuls
    per 512-column PSUM window - split by output-column parity pi, input
    pair lane e, and input superblock offset delta - at 4 diagonal
    tile positions (K=32) so the four 32-partition groups run
    concurrently in the PE array.
  - ScalarE squares PSUM->SBUF (bf16 out), GpSimd applies C - x with a
    per-partition constant, DVE stream-transposes back (f32), DMA out.
"""

import math

import numpy as np

import concourse.bass as bass
import concourse.tile as tile
from concourse import bacc, mybir
from concourse.bass_utils import run_bass_kernel_spmd

F32 = mybir.dt.float32
BF16 = mybir.dt.bfloat16
U32 = mybir.dt.uint32
P = 16  # AR order
HALO = 64  # bf16 halo = one 64-sample superblock (int32-pair aligned)

B_FULL, T_FULL = 256, 65536
N_CORES = 8

# MM kinds (pi, delta, e): output-column parity pi selects the psum bank,
# input pair-lane e and superblock offset delta select the rhs slice.
MM_KINDS = [
    (0, 0, 0), (0, 0, 1), (0, -1, 0), (0, -1, 1),  # accum group -> bank 0
    (1, 0, 0), (1, 0, 1),                          # accum group -> bank 1
]
N_TOEP = len(MM_KINDS)


def build_nc(b_core: int, t_len: int, rows_per_tile: int, win: int):
    R = rows_per_tile
    assert 128 % R == 0
    U = R * t_len // 128          # samples per partition per full tile
    cpr = 128 // R                # partitions per row
    assert cpr * U == t_len
    ntiles = b_core // R
    assert ntiles * R == b_core
    H = U // 2                    # half-tile samples per partition
    W = min(win, H)               # psum window width (f32 columns, 2 banks)
    assert H % W == 0 and W % 128 == 0
    nwin = H // W
    W2 = W // 2                   # per-parity bank width
    qper = W // 64                # superblocks per window

    nc = bacc.Bacc(
        "TRN2", target_bir_lowering=False, debug=False, enable_asserts=False
    )
    s_h = nc.declare_dram_parameter("s", [b_core, t_len], F32, isOutput=False)
    toep_h = nc.declare_dram_parameter(
        "toep", [128, 32 * N_TOEP], BF16, isOutput=False
    )
    cvec_h = nc.declare_dram_parameter("cvec", [128, 1], F32, isOutput=False)
    mask_h = nc.declare_dram_parameter("hmask", [128, 1], F32, isOutput=False)
    out_h = nc.declare_dram_parameter("out", [b_core, t_len], F32, isOutput=True)

    from contextlib import ExitStack

    with tile.TileContext(nc) as tc, ExitStack() as ctx:
        const_pool = ctx.enter_context(tc.tile_pool(name="const", bufs=1))
        in_pool = ctx.enter_context(tc.tile_pool(name="inp", bufs=4))
        st_pool = ctx.enter_context(tc.tile_pool(name="stp", bufs=3))
        sq_pool = ctx.enter_context(tc.tile_pool(name="sqp", bufs=3))
        out_pool = ctx.enter_context(tc.tile_pool(name="outp", bufs=3))
        psum_pool = ctx.enter_context(
            tc.tile_pool(name="psum", bufs=4, space="PSUM")
        )

        toep = const_pool.tile([128, 32 * N_TOEP], BF16)
        nc.sync.dma_start(out=toep[:, :], in_=toep_h.ap())
        cvec = const_pool.tile([128, 1], F32)
        nc.sync.dma_start(out=cvec[:, :], in_=cvec_h.ap())
        hmask = const_pool.tile([128, 1], F32)
        nc.sync.dma_start(out=hmask[:, :], in_=mask_h.ap())

        nhalves = ntiles * 2
        nats = [None] * nhalves

        def emit_input(t):
            g, h = divmod(t, 2)
            base = g * 128 * U + h * H  # flat sample offset of half-tile
            nat = in_pool.tile([128, H + HALO], BF16, tag="nat", name=f"nat{t}")
            if t == 0:
                Hq = H // 2
                q0_view = bass.AP(s_h, 0, [[U, 128], [1, Hq]])
                nc.gpsimd.dma_start(out=nat[:, HALO : HALO + Hq], in_=q0_view)
                halo_view = bass.AP(s_h, U - HALO, [[U, 127], [1, HALO]])
                nc.gpsimd.dma_start(out=nat[1:128, 0:HALO], in_=halo_view)
                q1_view = bass.AP(s_h, Hq - HALO, [[U, 128], [1, Hq + HALO]])
                nc.gpsimd.dma_start(
                    out=nat[:, Hq : H + HALO], in_=q1_view
                )
                nc.vector.memset(nat[0:1, 0:HALO], 0.0)
            else:
                ext_view = bass.AP(
                    s_h, base - HALO, [[U, 128], [1, H + HALO]]
                )
                nc.gpsimd.dma_start(out=nat[:, :], in_=ext_view)
            nats[t] = nat

        PREFETCH = 3
        for t in range(min(PREFETCH, nhalves)):
            emit_input(t)

        for g in range(ntiles):
            for h in range(2):
                base = g * 128 * U + h * H
                t = g * 2 + h
                nat = nats[t]
                if h == 0:
                    # zero the halo on row-start partitions
                    nc.vector.tensor_scalar_mul(
                        nat[:, 0:HALO], nat[:, 0:HALO], hmask[:, :]
                    )
                st = st_pool.tile([128, H + HALO], BF16, tag="st")
                if t == 0 and nwin == 2:
                    Hq = H // 2
                    nc.vector.transpose(
                        st.bitcast(U32)[:, : (Hq + HALO) // 2],
                        nat.bitcast(U32)[:, : (Hq + HALO) // 2],
                    )
                    nc.vector.transpose(
                        st.bitcast(U32)[:, (Hq + HALO) // 2 :],
                        nat.bitcast(U32)[:, (Hq + HALO) // 2 :],
                    )
                else:
                    nc.vector.transpose(
                        st.bitcast(U32)[:, :], nat.bitcast(U32)[:, :]
                    )

                sq = sq_pool.tile([128, H], BF16, tag="sq")
                for w in range(nwin):
                    q = psum_pool.tile([128, W], F32, tag="q")
                    for kidx, (pi, dlt, e) in enumerate(MM_KINDS):
                        s0 = 64 * (qper * w + dlt + 1) + e
                        for i in range(4):
                            pr = slice(32 * i, 32 * i + 32)
                            nc.tensor.matmul(
                                q[pr, pi * W2 : pi * W2 + W2],
                                toep[pr, 32 * kidx : 32 * kidx + 32],
                                st[pr, s0 : s0 + W - 1 : 2],
                                start=kidx in (0, 4),
                                stop=kidx in (3, 5),
                                tile_position=(32 * i, 32 * i),
                                skip_group_check=True,
                            )
                    # square PSUM->SBUF, permuting pi-major -> stream layout
                    nc.scalar.activation(
                        sq[:, w * W : (w + 1) * W].rearrange(
                            "p (Q pi b) -> p pi Q b", pi=2, b=32
                        ),
                        q.rearrange("p (pi Q b) -> p pi Q b", pi=2, b=32),
                        mybir.ActivationFunctionType.Square,
                    )

                if t + PREFETCH < nhalves:
                    emit_input(t + PREFETCH)
                aff = out_pool.tile([128, H], F32, tag="aff")
                nc.gpsimd.tensor_scalar(
                    aff[:, :],
                    sq[:, :],
                    -1.0,
                    cvec[:, :],
                    op0=mybir.AluOpType.mult,
                    op1=mybir.AluOpType.add,
                )
                onat = out_pool.tile([128, H], F32, tag="onat")
                nc.vector.transpose(onat[:, :], aff[:, :])
                out_view = bass.AP(out_h, base, [[U, 128], [1, H]])
                nc.sync.dma_start(out=out_view, in_=onat[:, :])

    nc.compile()
    return nc


def make_consts(coeffs: np.ndarray, noise_std: float):
    """Host-side O(1) prep: banded Toeplitz filter matrices + constants."""
    coeffs = np.asarray(coeffs, dtype=np.float64).reshape(-1)
    p = coeffs.shape[0]
    sigma = float(noise_std)
    invsc = 1.0 / (math.sqrt(2.0) * sigma)
    c_const = -0.5 * math.log(2.0 * math.pi * sigma * sigma)
    h = np.zeros(p + 1, dtype=np.float64)
    h[0] = -invsc
    h[1:] = invsc * coeffs

    mats = []
    for pi, dlt, e in MM_KINDS:
        # out time 64Q + 32*pi + m takes input pair-lane p (sample
        # 64(Q+dlt) + 2p + e): tap k = 32*pi + m - 2p - e - 64*dlt
        T = np.zeros((32, 32), dtype=np.float64)
        for pp in range(32):
            for m in range(32):
                k = 32 * pi + m - 2 * pp - e - 64 * dlt
                if 0 <= k <= p:
                    T[pp, m] = h[k]
        mats.append(T)
    import ml_dtypes

    toep = np.concatenate(mats, axis=1)                      # [32, 192]
    toep = np.tile(toep, (4, 1)).astype(ml_dtypes.bfloat16)  # [128, 192]
    cvec = np.full((128, 1), c_const, dtype=np.float32)
    return toep, cvec


def make_hmask(rows_per_tile: int) -> np.ndarray:
    cpr = 128 // rows_per_tile
    m = np.ones((128, 1), dtype=np.float32)
    m[::cpr] = 0.0
    return m


_NC_CACHE: dict = {}


def _get_nc(b_core, t_len, rows_per_tile=8, win=1024):
    key = (b_core, t_len, rows_per_tile, win)
    if key not in _NC_CACHE:
        _NC_CACHE[key] = build_nc(b_core, t_len, rows_per_tile, win)
    return _NC_CACHE[key]


def run_on_hw(s, coeffs, noise_std, rows_per_tile=8, win=1024, trace=False,
              tmpdir=None):
    """Shard across 8 cores, run, gather. Returns (out, BassKernelResults)."""
    s = np.ascontiguousarray(np.asarray(s, dtype=np.float32))
    b_full, t_len = s.shape
    b_core = b_full // N_CORES
    nc = _get_nc(b_core, t_len, rows_per_tile, win)
    toep, cvec = make_consts(coeffs, float(np.asarray(noise_std)))
    hmask = make_hmask(rows_per_tile)
    in_maps = [
        {
            "s": s[i * b_core : (i + 1) * b_core],
            "toep": toep,
            "cvec": cvec,
            "hmask": hmask,
        }
        for i in range(N_CORES)
    ]
    res = run_bass_kernel_spmd(
        nc, in_maps, core_ids=list(range(N_CORES)), trace=trace, tmpdir=tmpdir
    )
    out = np.concatenate([res.results[i]["out"] for i in range(N_CORES)], axis=0)
    return out, res


def kernel(s, coeffs, noise_std):
    out, _ = run_on_hw(s, coeffs, noise_std)
    return out


# revision 24
# speedup vs baseline: 1.0598x; 1.0598x over previous
"""AR(16) Gaussian log-likelihood kernel for Trainium2, 8 NeuronCores.

Math: out[b, t] = C - ((s[b,t] - sum_{k=1..16} phi_k s[b,t-k]) * invsc)^2
  with C = -0.5*log(2*pi*sigma^2), invsc = 1/(sqrt(2)*sigma).

Strategy (pure data parallel, 32 rows per core):
  - View each core's [32, 65536] shard as 8-row tiles laid out on 128
    SBUF partitions with U = 4096 contiguous samples per partition,
    processed in half-tiles of H = 2048 samples (+64-sample halo).
  - Input DMA casts f32->bf16 (SWDGE). DVE stream-transposes the bf16
    data viewed as int32 pairs (halves transpose cost): partition a of a
    32-group then holds the sample pairs (64j + 2a, 64j + 2a + 1).
  - TensorE computes q = (pred - s)*invsc with 6 banded-Toeplitz matmuls
    per 512-column PSUM window - split by output-column parity pi, input
    pair lane e, and input superblock offset delta - at 4 diagonal
    tile positions (K=32) so the four 32-partition groups run
    concurrently in the PE array.
  - ScalarE squares PSUM->SBUF (bf16 out), GpSimd applies C - x with a
    per-partition constant, DVE stream-transposes back (f32), DMA out.
"""

import math

import numpy as np

import concourse.bass as bass
import concourse.tile as tile
from concourse import bacc, mybir
from concourse.bass_utils import run_bass_kernel_spmd

F32 = mybir.dt.float32
BF16 = mybir.dt.bfloat16
U32 = mybir.dt.uint32
P = 16  # AR order
HALO = 64  # bf16 halo = one 64-sample superblock (int32-pair aligned)

B_FULL, T_FULL = 256, 65536
N_CORES = 8

# MM kinds (pi, delta, e): output-column parity pi selects the psum bank,
# input pair-lane e and superblock offset delta select the rhs slice.
MM_KINDS = [
    (0, 0, 0), (0, 0, 1), (0, -1, 0), (0, -1, 1),  # accum group -> bank 0
    (1, 0, 0), (1, 0, 1),                          # accum group -> bank 1
]
N_TOEP = len(MM_KINDS)


def build_nc(b_core: int, t_len: int, rows_per_tile: int, win: int):
    R = rows_per_tile
    assert 128 % R == 0
    U = R * t_len // 128          # samples per partition per full tile
    cpr = 128 // R                # partitions per row
    assert cpr * U == t_len
    ntiles = b_core // R
    assert ntiles * R == b_core
    H = U // 2                    # half-tile samples per partition
    W = min(win, H)               # psum window width (f32 columns, 2 banks)
    assert H % W == 0 and W % 128 == 0
    nwin = H // W
    W2 = W // 2                   # per-parity bank width
    qper = W // 64                # superblocks per window

    nc = bacc.Bacc(
        "TRN2", target_bir_lowering=False, debug=False, enable_asserts=False
    )
    s_h = nc.declare_dram_parameter("s", [b_core, t_len], F32, isOutput=False)
    toep_h = nc.declare_dram_parameter(
        "toep", [128, 32 * N_TOEP], BF16, isOutput=False
    )
    cvec_h = nc.declare_dram_parameter("cvec", [128, 1], F32, isOutput=False)
    mask_h = nc.declare_dram_parameter("hmask", [128, 1], F32, isOutput=False)
    out_h = nc.declare_dram_parameter("out", [b_core, t_len], F32, isOutput=True)

    from contextlib import ExitStack

    with tile.TileContext(nc) as tc, ExitStack() as ctx:
        const_pool = ctx.enter_context(tc.tile_pool(name="const", bufs=1))
        in_pool = ctx.enter_context(tc.tile_pool(name="inp", bufs=4))
        st_pool = ctx.enter_context(tc.tile_pool(name="stp", bufs=3))
        sq_pool = ctx.enter_context(tc.tile_pool(name="sqp", bufs=3))
        out_pool = ctx.enter_context(tc.tile_pool(name="outp", bufs=3))
        psum_pool = ctx.enter_context(
            tc.tile_pool(name="psum", bufs=4, space="PSUM")
        )

        toep = const_pool.tile([128, 32 * N_TOEP], BF16)
        nc.sync.dma_start(out=toep[:, :], in_=toep_h.ap())
        cvec = const_pool.tile([128, 1], F32)
        nc.sync.dma_start(out=cvec[:, :], in_=cvec_h.ap())
        hmask = const_pool.tile([128, 1], F32)
        nc.sync.dma_start(out=hmask[:, :], in_=mask_h.ap())

        nhalves = ntiles * 2
        nats = [None] * nhalves

        def emit_input(t):
            g, h = divmod(t, 2)
            base = g * 128 * U + h * H  # flat sample offset of half-tile
            nat = in_pool.tile([128, H + HALO], BF16, tag="nat", name=f"nat{t}")
            if t == 0:
                main_view = bass.AP(s_h, 0, [[U, 128], [1, H]])
                nc.gpsimd.dma_start(out=nat[:, HALO:], in_=main_view)
                halo_view = bass.AP(s_h, U - HALO, [[U, 127], [1, HALO]])
                nc.gpsimd.dma_start(out=nat[1:128, 0:HALO], in_=halo_view)
                nc.vector.memset(nat[0:1, 0:HALO], 0.0)
            else:
                ext_view = bass.AP(
                    s_h, base - HALO, [[U, 128], [1, H + HALO]]
                )
                nc.gpsimd.dma_start(out=nat[:, :], in_=ext_view)
            nats[t] = nat

        PREFETCH = 3
        for t in range(min(PREFETCH, nhalves)):
            emit_input(t)

        for g in range(ntiles):
            for h in range(2):
                base = g * 128 * U + h * H
                t = g * 2 + h
                nat = nats[t]
                if h == 0:
                    # zero the halo on row-start partitions
                    nc.vector.tensor_scalar_mul(
                        nat[:, 0:HALO], nat[:, 0:HALO], hmask[:, :]
                    )
                st = st_pool.tile([128, H + HALO], BF16, tag="st")
                nc.vector.transpose(
                    st.bitcast(U32)[:, :], nat.bitcast(U32)[:, :]
                )

                sq = sq_pool.tile([128, H], BF16, tag="sq")
                for w in range(nwin):
                    q = psum_pool.tile([128, W], F32, tag="q")
                    for kidx, (pi, dlt, e) in enumerate(MM_KINDS):
                        s0 = 64 * (qper * w + dlt + 1) + e
                        for i in range(4):
                            pr = slice(32 * i, 32 * i + 32)
                            nc.tensor.matmul(
                                q[pr, pi * W2 : pi * W2 + W2],
                                toep[pr, 32 * kidx : 32 * kidx + 32],
                                st[pr, s0 : s0 + W - 1 : 2],
                                start=kidx in (0, 4),
                                stop=kidx in (3, 5),
                                tile_position=(32 * i, 32 * i),
                                skip_group_check=True,
                            )
                    # square PSUM->SBUF, permuting pi-major -> stream layout
                    nc.scalar.activation(
                        sq[:, w * W : (w + 1) * W].rearrange(
                            "p (Q pi b) -> p pi Q b", pi=2, b=32
                        ),
                        q.rearrange("p (pi Q b) -> p pi Q b", pi=2, b=32),
                        mybir.ActivationFunctionType.Square,
                    )

                if t + PREFETCH < nhalves:
                    emit_input(t + PREFETCH)
                aff = out_pool.tile([128, H], F32, tag="aff")
                nc.gpsimd.tensor_scalar(
                    aff[:, :],
                    sq[:, :],
                    -1.0,
                    cvec[:, :],
                    op0=mybir.AluOpType.mult,
                    op1=mybir.AluOpType.add,
                )
                onat = out_pool.tile([128, H], F32, tag="onat")
                nc.vector.transpose(onat[:, :], aff[:, :])
                out_view = bass.AP(out_h, base, [[U, 128], [1, H]])
                nc.sync.dma_start(out=out_view, in_=onat[:, :])

    nc.compile()
    return nc


def make_consts(coeffs: np.ndarray, noise_std: float):
    """Host-side O(1) prep: banded Toeplitz filter matrices + constants."""
    coeffs = np.asarray(coeffs, dtype=np.float64).reshape(-1)
    p = coeffs.shape[0]
    sigma = float(noise_std)
    invsc = 1.0 / (math.sqrt(2.0) * sigma)
    c_const = -0.5 * math.log(2.0 * math.pi * sigma * sigma)
    h = np.zeros(p + 1, dtype=np.float64)
    h[0] = -invsc
    h[1:] = invsc * coeffs

    mats = []
    for pi, dlt, e in MM_KINDS:
        # out time 64Q + 32*pi + m takes input pair-lane p (sample
        # 64(Q+dlt) + 2p + e): tap k = 32*pi + m - 2p - e - 64*dlt
        T = np.zeros((32, 32), dtype=np.float64)
        for pp in range(32):
            for m in range(32):
                k = 32 * pi + m - 2 * pp - e - 64 * dlt
                if 0 <= k <= p:
                    T[pp, m] = h[k]
        mats.append(T)
    import ml_dtypes

    toep = np.concatenate(mats, axis=1)                      # [32, 192]
    toep = np.tile(toep, (4, 1)).astype(ml_dtypes.bfloat16)  # [128, 192]
    cvec = np.full((128, 1), c_const, dtype=np.float32)
    return toep, cvec


def make_hmask(rows_per_tile: int) -> np.ndarray:
    cpr = 128 // rows_per_tile
    m = np.ones((128, 1), dtype=np.float32)
    m[::cpr] = 0.0
    return m


_NC_CACHE: dict = {}


def _get_nc(b_core, t_len, rows_per_tile=8, win=1024):
    key = (b_core, t_len, rows_per_tile, win)
    if key not in _NC_CACHE:
        _NC_CACHE[key] = build_nc(b_core, t_len, rows_per_tile, win)
    return _NC_CACHE[key]


def run_on_hw(s, coeffs, noise_std, rows_per_tile=8, win=1024, trace=False,
              tmpdir=None):
    """Shard across 8 cores, run, gather. Returns (out, BassKernelResults)."""
    s = np.ascontiguousarray(np.asarray(s, dtype=np.float32))
    b_full, t_len = s.shape
    b_core = b_full // N_CORES
    nc = _get_nc(b_core, t_len, rows_per_tile, win)
    toep, cvec = make_consts(coeffs, float(np.asarray(noise_std)))
    hmask = make_hmask(rows_per_tile)
    in_maps = [
        {
            "s": s[i * b_core : (i + 1) * b_core],
            "toep": toep,
            "cvec": cvec,
            "hmask": hmask,
        }
        for i in range(N_CORES)
    ]
    res = run_bass_kernel_spmd(
        nc, in_maps, core_ids=list(range(N_CORES)), trace=trace, tmpdir=tmpdir
    )
    out = np.concatenate([res.results[i]["out"] for i in range(N_CORES)], axis=0)
    return out, res


def kernel(s, coeffs, noise_std):
    out, _ = run_on_hw(s, coeffs, noise_std)
    return out
